# revision 6
# baseline (speedup 1.0000x reference)
"""BertLinearCRF kernel for 8 Trainium2 NeuronCores.

Strategy (data-parallel over batch, 4 sequences/core):
  - Emissions: label-major PE matmul  em[(b,l), s] = sum_h W[l,h] hs[b,s,h],
    host pre-transposes hidden_states to [H, 4*S] per core for DMA-friendly
    layout. PE-transpose tiles produce the token-major [4*S, L] DRAM output.
  - CRF log-partition: exp-domain associative scan. The per-step transfer
    matrix factors as (expT @ diag(w_t)) with expT shared by every step, so
    each sequence's 511-step chain is chunked (K=16 chunks x C=32 steps);
    all chunks advance together with one blockdiag(expT x4) matmul per step,
    then a DVE multiply by the w-column broadcast. Sequence blocks sit at
    partition bases {0,32,64,96} (PE quadrant rule); gap rows are kept zero.
    Stability: constant 2^-4 folded into exp's bias (exact in f32), chunk
    matrices sum-normalized (mass output, logs re-added on host).
  - Phase B combines the 16 chunk matrices per sequence with blockdiag
    matmuls (Racc <- A_k @ Racc, k = 15..0).
  - Host (f64): gold-path numerator via indexing over the emissions output,
    partition = log(sum racc * exp(end)) + offsets, loss = -mean(llh).
"""
import math
import numpy as np
import sys

sys.path.insert(0, "/opt/trn_rl_repo")

B, S, H, L = 32, 512, 1024, 17
NCORES = 8
BL = B // NCORES          # sequences per core = 4
PB = 32                   # partition stride between sequence blocks
K = 16                    # chunks per sequence
C = S // K                # 32 scan steps
LN2 = math.log(2.0)
KH = H // 128             # 8 contraction tiles

_cache: dict = {}


def _build_program():
    import concourse.bacc as bacc
    import concourse.tile as tile
    from concourse import mybir
    import concourse.bass as bass

    f32 = mybir.dt.float32
    nc = bacc.Bacc("TRN2", target_bir_lowering=False, debug=False)

    t_hsT = nc.dram_tensor("hsT", [H, BL * S], f32, kind="ExternalInput")
    t_wt = nc.dram_tensor("wt", [H, L], f32, kind="ExternalInput")
    t_bias4 = nc.dram_tensor("bias4", [128, 1], f32, kind="ExternalInput")
    t_bias0 = nc.dram_tensor("bias0", [128, 1], f32, kind="ExternalInput")
    t_expT128 = nc.dram_tensor("expT128", [128, 128], f32, kind="ExternalInput")
    t_init128 = nc.dram_tensor("init128", [128, K * L], f32, kind="ExternalInput")
    t_ident128 = nc.dram_tensor("ident128", [128, L], f32, kind="ExternalInput")
    t_b684 = nc.dram_tensor("b684", [128, BL], f32, kind="ExternalInput")
    t_b468 = nc.dram_tensor("b468", [BL, 128], f32, kind="ExternalInput")
    t_emout = nc.dram_tensor("emissions", [BL * S, L], f32, kind="ExternalOutput")
    t_racc = nc.dram_tensor("racc", [128, L], f32, kind="ExternalOutput")
    t_mass = nc.dram_tensor("mass", [BL, K], f32, kind="ExternalOutput")

    with tile.TileContext(nc) as tc:
        with (
            tc.tile_pool(name="consts", bufs=1) as consts,
            tc.tile_pool(name="big", bufs=1) as big,
            tc.tile_pool(name="work", bufs=2) as work,
            tc.tile_pool(name="emo", bufs=4) as emo,
            tc.tile_pool(name="ps", bufs=2, space=bass.MemorySpace.PSUM) as ps,
            tc.tile_pool(name="ps_s", bufs=2, space=bass.MemorySpace.PSUM) as ps_s,
        ):
            # ---- constants ----
            wt_sb = consts.tile([128, KH, L], f32)
            nc.sync.dma_start(wt_sb, t_wt.rearrange("(k p) l -> p k l", p=128))
            bias4_sb = consts.tile([128, 1], f32)
            nc.sync.dma_start(bias4_sb, t_bias4[:, :])
            bias0_sb = consts.tile([128, 1], f32)
            nc.sync.dma_start(bias0_sb, t_bias0[:, :])
            expT_sb = consts.tile([128, 128], f32)
            nc.sync.dma_start(expT_sb, t_expT128[:, :])
            ident_sb = consts.tile([128, L], f32)
            nc.sync.dma_start(ident_sb, t_ident128[:, :])
            b684_sb = consts.tile([128, BL], f32)
            nc.sync.dma_start(b684_sb, t_b684[:, :])
            b468_sb = consts.tile([BL, 128], f32)
            nc.sync.dma_start(b468_sb, t_b468[:, :])

            # ---- hidden states [H, BL*S], 8 x 1MB DMA ----
            hsT_sb = big.tile([128, KH, BL * S], f32)
            for k8 in range(KH):
                nc.sync.dma_start(
                    hsT_sb[:, k8, :], t_hsT[k8 * 128 : (k8 + 1) * 128, :]
                )

            # ---- emissions matmul into gapped psum [128, S] ----
            psum128 = ps.tile([128, S], f32, tag="em")
            nc.vector.memset(psum128, 0.0)      # keep gap rows exactly zero
            for b in range(BL):
                for k8 in range(KH):
                    nc.tensor.matmul(
                        psum128[b * PB : b * PB + L, :],
                        wt_sb[:, k8, :],
                        hsT_sb[:, k8, b * S : (b + 1) * S],
                        start=(k8 == 0),
                        stop=(k8 == KH - 1),
                        tile_position=(0, b * PB),
                    )

            # w128 = exp(em + b - 4ln2); em128 = em + b
            w128 = big.tile([128, S], f32)
            nc.scalar.activation(
                w128, psum128, mybir.ActivationFunctionType.Exp, bias=bias4_sb
            )
            em128 = big.tile([128, S], f32)
            nc.vector.tensor_scalar_add(em128, psum128, bias0_sb)

            # ---- emissions output: PE-transpose [17,128] -> [128,17] ----
            for b in range(BL):
                for q in range(S // 128):
                    ps_t = ps_s.tile([128, L], f32, tag="tr")
                    nc.tensor.transpose(
                        ps_t,
                        em128[b * PB : b * PB + L, q * 128 : (q + 1) * 128],
                        ident_sb[b * PB : b * PB + L, :],
                        tile_position=(b * PB, 0),
                    )
                    ot = emo.tile([128, L], f32, tag="ot")
                    nc.vector.tensor_copy(ot, ps_t)
                    nc.sync.dma_start(
                        t_emout[b * S + q * 128 : b * S + (q + 1) * 128, :], ot
                    )

            # ---- Phase A: chunked scan ----
            Ra = big.tile([128, K, L], f32)
            Rb = big.tile([128, K, L], f32)
            nc.sync.dma_start(Ra, t_init128.rearrange("p (k x) -> p k x", x=L))
            R = [Ra, Rb]
            for c in range(C):
                src, dst = R[c % 2], R[(c + 1) % 2]
                nk = K if c < C - 1 else K - 1
                pa = ps.tile([128, K, L], f32, tag="pa")
                nc.tensor.matmul(pa[:, :nk, :], expT_sb, src[:, :nk, :])
                base = w128[:, c + 1 : c + 2]
                w_ap = bass.AP(
                    tensor=base.tensor,
                    offset=base.offset,
                    ap=[base.ap[0], [C, nk], [0, L]],
                )
                nc.vector.tensor_mul(dst[:, :nk, :], pa[:, :nk, :], w_ap)
                if nk < K:
                    nc.vector.tensor_copy(dst[:, nk:, :], src[:, nk:, :])
            Rfin = R[C % 2]
            Rnorm = R[(C + 1) % 2]

            # ---- mass + normalize ----
            pm = ps.tile([BL, K, L], f32, tag="em")
            nc.tensor.matmul(pm, b684_sb, Rfin)
            s16 = work.tile([BL, K], f32, tag="s16")
            nc.vector.reduce_sum(
                s16, pm, axis=mybir.AxisListType.X, op=mybir.AluOpType.add
            )
            nc.sync.dma_start(t_mass[:, :], s16)
            rec = work.tile([BL, K], f32, tag="rec")
            nc.vector.reciprocal(rec, s16)
            pr = ps.tile([128, K], f32, tag="pa")
            nc.tensor.matmul(pr, b468_sb, rec)
            rec128 = work.tile([128, K], f32, tag="rec128")
            nc.vector.tensor_copy(rec128, pr)
            rbase = rec128[:, 0:1]
            rec_ap = bass.AP(
                tensor=rbase.tensor,
                offset=rbase.offset,
                ap=[rbase.ap[0], [1, K], [0, L]],
            )
            nc.vector.tensor_mul(Rnorm, Rfin, rec_ap)

            # ---- Phase B: Racc <- A_k @ Racc, k = K-1..0 ----
            bd = big.tile([128, 128], f32)
            nc.vector.memset(bd, 0.0)
            r0 = work.tile([128, L], f32, tag="r0")
            r1 = work.tile([128, L], f32, tag="r1")
            nc.sync.dma_start(r0, t_ident128[:, :])
            rp = [r0, r1]
            for i, k in enumerate(range(K - 1, -1, -1)):
                src, dst = rp[i % 2], rp[(i + 1) % 2]
                for b in range(BL):
                    nc.vector.tensor_copy(
                        bd[b * PB : b * PB + L, b * PB : b * PB + L],
                        Rnorm[b * PB : b * PB + L, k, :],
                    )
                pb = ps_s.tile([128, L], f32, tag="pb")
                nc.tensor.matmul(pb, bd, src)
                nc.vector.tensor_copy(dst, pb)
            nc.sync.dma_start(t_racc[:, :], rp[K % 2])

    nc.compile()
    return nc


def _host_constants(hs, W, b, start_transitions, transitions):
    """Per-core input maps (device consts + sharded hsT) and c1 offsets."""
    f32 = np.float32
    wt = np.ascontiguousarray(W.T).astype(f32)                      # [H, L]
    expT = np.exp(transitions.astype(np.float64)).astype(f32)
    expT128 = np.zeros((128, 128), f32)
    bias4 = np.zeros((128, 1), f32)
    bias0 = np.zeros((128, 1), f32)
    ident128 = np.zeros((128, L), f32)
    b684 = np.zeros((128, BL), f32)
    b468 = np.zeros((BL, 128), f32)
    for b_ in range(BL):
        r = b_ * PB
        expT128[r : r + L, r : r + L] = expT
        bias4[r : r + L, 0] = b - 4 * LN2
        bias0[r : r + L, 0] = b
        ident128[r : r + L, :] = np.eye(L, dtype=f32)
        b684[r : r + L, b_] = 1.0 / 16.0
        b468[b_, r : r + L] = 1.0
    bias4[:, 0] = np.where(bias4[:, 0] == 0.0, -4 * LN2, bias4[:, 0])

    em0 = (hs[:, 0, :].astype(f32) @ W.T.astype(f32) + b).astype(f32)   # [B, L]
    alpha1 = start_transitions[None, :].astype(f32) + em0
    c1 = alpha1.max(1)                                              # [B]
    v1 = np.exp((alpha1 - c1[:, None]).astype(f32)).astype(f32)

    in_maps = []
    for core in range(NCORES):
        sl = slice(core * BL, (core + 1) * BL)
        hsT = np.ascontiguousarray(
            hs[sl].reshape(BL * S, H).T
        ).astype(f32)                                               # [H, BL*S]
        init128 = np.zeros((128, K * L), f32)
        for b_ in range(BL):
            blk = init128[b_ * PB : b_ * PB + L]
            for k_ in range(1, K):
                blk[:, k_ * L : (k_ + 1) * L] = np.eye(L, dtype=f32)
            blk[:, 0:L] = np.diag(v1[core * BL + b_])
        in_maps.append(
            {
                "hsT": hsT,
                "wt": wt,
                "bias4": bias4,
                "bias0": bias0,
                "expT128": expT128,
                "init128": init128,
                "ident128": ident128,
                "b684": b684,
                "b468": b468,
            }
        )
    return in_maps, c1


def _run(inputs, trace=False):
    from concourse.bass_utils import run_bass_kernel_spmd

    hs = np.asarray(inputs["hidden_states"], np.float32)
    labels = np.asarray(inputs["labels"])
    W = np.asarray(inputs["W"], np.float32)
    b = np.asarray(inputs["b"], np.float32)
    st = np.asarray(inputs["start_transitions"], np.float32)
    en = np.asarray(inputs["end_transitions"], np.float32)
    T = np.asarray(inputs["transitions"], np.float32)

    if "nc" not in _cache:
        _cache["nc"] = _build_program()
    nc = _cache["nc"]

    in_maps, c1 = _host_constants(hs, W, b, st, T)
    res = run_bass_kernel_spmd(
        nc, in_maps, core_ids=list(range(NCORES)), trace=trace
    )

    # ---- host assembly (f64) ----
    em = np.concatenate(
        [r["emissions"].reshape(BL, S, L) for r in res.results], axis=0
    ).astype(np.float32)                                            # [B, S, L]
    racc_g = np.stack([r["racc"] for r in res.results], axis=0)     # [8, 128, L]
    racc = np.zeros((B, L, L), np.float64)
    for core in range(NCORES):
        for b_ in range(BL):
            racc[core * BL + b_] = racc_g[core, b_ * PB : b_ * PB + L, :]
    s16 = np.concatenate([r["mass"] for r in res.results], axis=0).astype(
        np.float64
    )                                                               # [B, K]

    vS = racc.sum(axis=1)                                           # [B, L]
    partition = (
        np.log((vS * np.exp(en.astype(np.float64))[None]).sum(1))
        + c1.astype(np.float64)
        + (S - 1) * 4 * LN2
        + np.log(s16).sum(1)
    )
    tags = labels.astype(np.int64)
    em64 = em.astype(np.float64)
    emit = np.take_along_axis(em64, tags[:, :, None], axis=2)[:, :, 0]
    trans_sc = T.astype(np.float64)[tags[:, :-1], tags[:, 1:]]
    numerator = (
        st.astype(np.float64)[tags[:, 0]]
        + emit[:, 0]
        + (trans_sc + emit[:, 1:]).sum(1)
        + en.astype(np.float64)[tags[:, -1]]
    )
    llh = numerator - partition
    loss = np.float32(-llh.mean())
    return (loss, em), res


def kernel(**inputs):
    (loss, em), _ = _run(inputs, trace=False)
    return loss, em


# revision 14
# speedup vs baseline: 1.1607x; 1.1607x over previous
"""BertLinearCRF kernel for 8 Trainium2 NeuronCores.

Strategy (data-parallel over batch, 4 sequences/core):
  - Emissions: label-major PE matmul  em[(b,l), s] = sum_h W[l,h] hs[b,s,h],
    host pre-transposes hidden_states to [H, 4*S] per core for DMA-friendly
    layout. PE-transpose tiles produce the token-major [4*S, L] DRAM output.
  - CRF log-partition: exp-domain associative scan. The per-step transfer
    matrix factors as (expT @ diag(w_t)) with expT shared by every step, so
    each sequence's 511-step chain is chunked (K=16 chunks x C=32 steps);
    all chunks advance together with one blockdiag(expT x4) matmul per step,
    then a DVE multiply by the w-column broadcast. Sequence blocks sit at
    partition bases {0,32,64,96} (PE quadrant rule); gap rows are kept zero.
    Stability: constant 2^-4 folded into exp's bias (exact in f32), chunk
    matrices sum-normalized (mass output, logs re-added on host).
  - Phase B combines the 16 chunk matrices per sequence with blockdiag
    matmuls (Racc <- A_k @ Racc, k = 15..0).
  - Host (f64): gold-path numerator via indexing over the emissions output,
    partition = log(sum racc * exp(end)) + offsets, loss = -mean(llh).
"""
import math
import numpy as np
import sys

sys.path.insert(0, "/opt/trn_rl_repo")

B, S, H, L = 32, 512, 1024, 17
NCORES = 8
BL = B // NCORES          # sequences per core = 4
PB = 32                   # partition stride between sequence blocks
K = 16                    # chunks per sequence
C = S // K                # 32 scan steps
LN2 = math.log(2.0)
KH = H // 128             # 8 contraction tiles

_cache: dict = {}


def _build_program(em=True, emout=True, scan=True, phaseb=True):
    import concourse.bacc as bacc
    import concourse.tile as tile
    from concourse import mybir
    import concourse.bass as bass

    f32 = mybir.dt.float32
    f32r = mybir.dt.float32r
    nc = bacc.Bacc("TRN2", target_bir_lowering=False, debug=False)

    t_hsT = nc.dram_tensor("hsT", [H, BL * S], f32r, kind="ExternalInput")
    t_wt = nc.dram_tensor("wt", [H, L], f32r, kind="ExternalInput")
    t_bias4 = nc.dram_tensor("bias4", [128, 1], f32, kind="ExternalInput")
    t_bias0 = nc.dram_tensor("bias0", [128, 1], f32, kind="ExternalInput")
    t_expT128 = nc.dram_tensor("expT128", [128, 128], f32r, kind="ExternalInput")
    t_init128 = nc.dram_tensor("init128", [128, K * L], f32r, kind="ExternalInput")
    t_ident128 = nc.dram_tensor("ident128", [128, L], f32, kind="ExternalInput")
    t_b684 = nc.dram_tensor("b684", [128, BL], f32r, kind="ExternalInput")
    t_b468 = nc.dram_tensor("b468", [BL, 128], f32r, kind="ExternalInput")
    t_emout = nc.dram_tensor("emissions", [BL * S, L], f32, kind="ExternalOutput")
    t_racc = nc.dram_tensor("racc", [128, L], f32, kind="ExternalOutput")
    t_mass = nc.dram_tensor("mass", [BL, K], f32, kind="ExternalOutput")

    with tile.TileContext(nc) as tc:
        with (
            tc.tile_pool(name="consts", bufs=1) as consts,
            tc.tile_pool(name="big", bufs=1) as big,
            tc.tile_pool(name="work", bufs=2) as work,
            tc.tile_pool(name="emo", bufs=4) as emo,
            tc.tile_pool(name="ps", bufs=1, space=bass.MemorySpace.PSUM) as ps,
            tc.tile_pool(name="ps_s", bufs=2, space=bass.MemorySpace.PSUM) as ps_s,
        ):
            # ---- constants ----
            wt_sb = consts.tile([128, KH, L], f32r)
            nc.sync.dma_start(wt_sb, t_wt.rearrange("(k p) l -> p k l", p=128))
            bias4_sb = consts.tile([128, 1], f32)
            nc.sync.dma_start(bias4_sb, t_bias4[:, :])
            bias0_sb = consts.tile([128, 1], f32)
            nc.sync.dma_start(bias0_sb, t_bias0[:, :])
            expT_sb = consts.tile([128, 128], f32r)
            nc.sync.dma_start(expT_sb, t_expT128[:, :])
            ident_sb = consts.tile([128, L], f32)
            nc.sync.dma_start(ident_sb, t_ident128[:, :])
            b684_sb = consts.tile([128, BL], f32r)
            nc.sync.dma_start(b684_sb, t_b684[:, :])
            b468_sb = consts.tile([BL, 128], f32r)
            nc.sync.dma_start(b468_sb, t_b468[:, :])

            # ---- hidden states [H, BL*S], 8 x 1MB DMA ----
            hsT_sb = big.tile([128, KH, BL * S], f32r)
            for k8 in range(KH):
                nc.sync.dma_start(
                    hsT_sb[:, k8, :], t_hsT[k8 * 128 : (k8 + 1) * 128, :]
                )

            # ---- emissions matmul: psum [17, 4, 512] (bank per seq) ----
            psum_em = ps.tile([L, BL, S], f32, tag="em")
            for b in range(BL):
                for k8 in range(KH):
                    nc.tensor.matmul(
                        psum_em[:, b, :],
                        wt_sb[:, k8, :],
                        hsT_sb[:, k8, b * S : (b + 1) * S],
                        start=(k8 == 0),
                        stop=(k8 == KH - 1),
                    )

            # w128 = exp(em + b - 4ln2); em128 = em + b  (partition-shifted
            # per-sequence: psum rows 0-16 -> sbuf rows 32b..32b+16)
            w128 = big.tile([128, S], f32)
            em128 = big.tile([128, S], f32)
            for b in range(BL):
                nc.scalar.activation(
                    w128[b * PB : b * PB + L, :],
                    psum_em[:, b, :],
                    mybir.ActivationFunctionType.Exp,
                    bias=bias4_sb[b * PB : b * PB + L, :],
                )
                nc.vector.tensor_scalar_add(
                    em128[b * PB : b * PB + L, :],
                    psum_em[:, b, :],
                    bias0_sb[b * PB : b * PB + L, :],
                )

            # ---- emissions output: PE-transpose [17,128] -> [128,17] ----
            for b in range(BL if emout else 0):
                for q in range(S // 128):
                    ps_t = ps_s.tile([128, L], f32, tag="pa")
                    nc.tensor.transpose(
                        ps_t,
                        em128[b * PB : b * PB + L, q * 128 : (q + 1) * 128],
                        ident_sb[b * PB : b * PB + L, :],
                        tile_position=(b * PB, 0),
                    )
                    ot = emo.tile([128, L], f32, tag="ot")
                    nc.vector.tensor_copy(ot, ps_t)
                    nc.sync.dma_start(
                        t_emout[b * S + q * 128 : b * S + (q + 1) * 128, :], ot
                    )

            # ---- Phase A: chunked scan ----
            Ra = big.tile([128, K, L], f32r)
            Rb = big.tile([128, K, L], f32r)
            nc.sync.dma_start(Ra, t_init128.rearrange("p (k x) -> p k x", x=L))
            R = [Ra, Rb]
            for c in range(C if scan else 0):
                src, dst = R[c % 2], R[(c + 1) % 2]
                nk = K if c < C - 1 else K - 1
                pa = ps_s.tile([128, K, L], f32, tag="pa")
                nc.tensor.matmul(pa[:, :, :], expT_sb[:, :], src[:, :, :])
                base = w128[:, c + 1 : c + 2]
                w_ap = bass.AP(
                    tensor=base.tensor,
                    offset=base.offset,
                    ap=[base.ap[0], [C, nk], [0, L]],
                )
                nc.vector.tensor_mul(dst[:, :nk, :], pa[:, :nk, :], w_ap)
                if nk < K:
                    nc.vector.tensor_copy(dst[:, nk:, :], src[:, nk:, :])
            Rfin = R[C % 2]
            Rnorm = R[(C + 1) % 2]

            # ---- mass + normalize ----
            pm = ps_s.tile([BL, K, L], f32, tag="pa")
            nc.tensor.matmul(pm, b684_sb[:, :], Rfin[:, :, :])
            s16 = work.tile([BL, K], f32, tag="s16")
            nc.vector.reduce_sum(
                s16, pm, axis=mybir.AxisListType.X, op=mybir.AluOpType.add
            )
            nc.sync.dma_start(t_mass[:, :], s16)
            rec = work.tile([BL, K], f32, tag="rec")
            nc.vector.reciprocal(rec, s16)
            pr = ps_s.tile([128, K], f32, tag="pa")
            nc.tensor.matmul(pr, b468_sb[:, :].bitcast(f32), rec[:, :])
            rec128 = work.tile([128, K], f32, tag="rec128")
            nc.vector.tensor_copy(rec128, pr)
            rbase = rec128[:, 0:1]
            rec_ap = bass.AP(
                tensor=rbase.tensor,
                offset=rbase.offset,
                ap=[rbase.ap[0], [1, K], [0, L]],
            )
            nc.vector.tensor_mul(Rnorm, Rfin, rec_ap)

            # ---- Phase B: Racc <- A_k @ Racc, k = K-1..0 ----
            bd = big.tile([128, 128], f32)
            nc.vector.memset(bd, 0.0)
            r0 = work.tile([128, L], f32, tag="r0")
            r1 = work.tile([128, L], f32, tag="r1")
            nc.vector.tensor_copy(r0, ident_sb[:, :])
            rp = [r0, r1]
            for i, k in enumerate(range(K - 1, -1, -1) if phaseb else []):
                src, dst = rp[i % 2], rp[(i + 1) % 2]
                for b in range(BL):
                    nc.vector.tensor_copy(
                        bd[b * PB : b * PB + L, b * PB : b * PB + L],
                        Rnorm[b * PB : b * PB + L, k, :],
                    )
                pb = ps_s.tile([128, L], f32, tag="pb")
                nc.tensor.matmul(pb, bd[:, :], src[:, :])
                nc.vector.tensor_copy(dst, pb)
            nc.sync.dma_start(t_racc[:, :], rp[K % 2])

    nc.compile()
    return nc


def _host_constants(hs, W, b, start_transitions, transitions):
    """Per-core input maps (device consts + sharded hsT) and c1 offsets."""
    f32 = np.float32
    wt = np.ascontiguousarray(W.T).astype(f32)                      # [H, L]
    expT = np.exp(transitions.astype(np.float64)).astype(f32)
    expT128 = np.zeros((128, 128), f32)
    bias4 = np.zeros((128, 1), f32)
    bias0 = np.zeros((128, 1), f32)
    ident128 = np.zeros((128, L), f32)
    b684 = np.zeros((128, BL), f32)
    b468 = np.zeros((BL, 128), f32)
    for b_ in range(BL):
        r = b_ * PB
        expT128[r : r + L, r : r + L] = expT
        bias4[r : r + L, 0] = b - 4 * LN2
        bias0[r : r + L, 0] = b
        ident128[r : r + L, :] = np.eye(L, dtype=f32)
        b684[r : r + L, b_] = 1.0 / 16.0
        b468[b_, r : r + L] = 1.0
    bias4[:, 0] = np.where(bias4[:, 0] == 0.0, -4 * LN2, bias4[:, 0])

    em0 = (hs[:, 0, :].astype(f32) @ W.T.astype(f32) + b).astype(f32)   # [B, L]
    alpha1 = start_transitions[None, :].astype(f32) + em0
    c1 = alpha1.max(1)                                              # [B]
    v1 = np.exp((alpha1 - c1[:, None]).astype(f32)).astype(f32)

    in_maps = []
    for core in range(NCORES):
        sl = slice(core * BL, (core + 1) * BL)
        hsT = np.ascontiguousarray(
            hs[sl].reshape(BL * S, H).T
        ).astype(f32)                                               # [H, BL*S]
        init128 = np.zeros((128, K * L), f32)
        for b_ in range(BL):
            blk = init128[b_ * PB : b_ * PB + L]
            for k_ in range(1, K):
                blk[:, k_ * L : (k_ + 1) * L] = np.eye(L, dtype=f32)
            blk[:, 0:L] = np.diag(v1[core * BL + b_])
        in_maps.append(
            {
                "hsT": hsT,
                "wt": wt,
                "bias4": bias4,
                "bias0": bias0,
                "expT128": expT128,
                "init128": init128,
                "ident128": ident128,
                "b684": b684,
                "b468": b468,
            }
        )
    return in_maps, c1


def _run(inputs, trace=False):
    from concourse.bass_utils import run_bass_kernel_spmd

    hs = np.asarray(inputs["hidden_states"], np.float32)
    labels = np.asarray(inputs["labels"])
    W = np.asarray(inputs["W"], np.float32)
    b = np.asarray(inputs["b"], np.float32)
    st = np.asarray(inputs["start_transitions"], np.float32)
    en = np.asarray(inputs["end_transitions"], np.float32)
    T = np.asarray(inputs["transitions"], np.float32)

    if "nc" not in _cache:
        _cache["nc"] = _build_program()
    nc = _cache["nc"]

    in_maps, c1 = _host_constants(hs, W, b, st, T)
    res = run_bass_kernel_spmd(
        nc, in_maps, core_ids=list(range(NCORES)), trace=trace
    )

    # ---- host assembly (f64) ----
    em = np.concatenate(
        [r["emissions"].reshape(BL, S, L) for r in res.results], axis=0
    ).astype(np.float32)                                            # [B, S, L]
    racc_g = np.stack([r["racc"] for r in res.results], axis=0)     # [8, 128, L]
    racc = np.zeros((B, L, L), np.float64)
    for core in range(NCORES):
        for b_ in range(BL):
            racc[core * BL + b_] = racc_g[core, b_ * PB : b_ * PB + L, :]
    s16 = np.concatenate([r["mass"] for r in res.results], axis=0).astype(
        np.float64
    )                                                               # [B, K]

    vS = racc.sum(axis=1)                                           # [B, L]
    partition = (
        np.log((vS * np.exp(en.astype(np.float64))[None]).sum(1))
        + c1.astype(np.float64)
        + (S - 1) * 4 * LN2
        + np.log(s16).sum(1)
    )
    tags = labels.astype(np.int64)
    em64 = em.astype(np.float64)
    emit = np.take_along_axis(em64, tags[:, :, None], axis=2)[:, :, 0]
    trans_sc = T.astype(np.float64)[tags[:, :-1], tags[:, 1:]]
    numerator = (
        st.astype(np.float64)[tags[:, 0]]
        + emit[:, 0]
        + (trans_sc + emit[:, 1:]).sum(1)
        + en.astype(np.float64)[tags[:, -1]]
    )
    llh = numerator - partition
    loss = np.float32(-llh.mean())
    return (loss, em), res


def kernel(**inputs):
    (loss, em), _ = _run(inputs, trace=False)
    return loss, em


# revision 15
# speedup vs baseline: 1.3065x; 1.1256x over previous
"""BertLinearCRF kernel for 8 Trainium2 NeuronCores.

Strategy (data-parallel over batch, 4 sequences/core):
  - Emissions: label-major PE matmul  em[(b,l), s] = sum_h W[l,h] hs[b,s,h],
    host pre-transposes hidden_states to [H, 4*S] per core for DMA-friendly
    layout. PE-transpose tiles produce the token-major [4*S, L] DRAM output.
  - CRF log-partition: exp-domain associative scan. The per-step transfer
    matrix factors as (expT @ diag(w_t)) with expT shared by every step, so
    each sequence's 511-step chain is chunked (K=16 chunks x C=32 steps);
    all chunks advance together with one blockdiag(expT x4) matmul per step,
    then a DVE multiply by the w-column broadcast. Sequence blocks sit at
    partition bases {0,32,64,96} (PE quadrant rule); gap rows are kept zero.
    Stability: constant 2^-4 folded into exp's bias (exact in f32), chunk
    matrices sum-normalized (mass output, logs re-added on host).
  - Phase B combines the 16 chunk matrices per sequence with blockdiag
    matmuls (Racc <- A_k @ Racc, k = 15..0).
  - Host (f64): gold-path numerator via indexing over the emissions output,
    partition = log(sum racc * exp(end)) + offsets, loss = -mean(llh).
"""
import math
import numpy as np
import sys

sys.path.insert(0, "/opt/trn_rl_repo")

B, S, H, L = 32, 512, 1024, 17
NCORES = 8
BL = B // NCORES          # sequences per core = 4
PB = 32                   # partition stride between sequence blocks
K = 16                    # chunks per sequence
C = S // K                # 32 scan steps
LN2 = math.log(2.0)
KH = H // 128             # 8 contraction tiles

_cache: dict = {}


def _build_program(em=True, emout=True, scan=True, phaseb=True):
    import concourse.bacc as bacc
    import concourse.tile as tile
    from concourse import mybir
    import concourse.bass as bass

    f32 = mybir.dt.float32
    f32r = mybir.dt.float32r
    nc = bacc.Bacc("TRN2", target_bir_lowering=False, debug=False)

    HS = S // 2                # 256 tokens per half
    KHALF = K // 2             # 8 chunks per half

    t_hsT = nc.dram_tensor("hsT", [H, BL * S], f32r, kind="ExternalInput")
    t_wt = nc.dram_tensor("wt", [H, L], f32r, kind="ExternalInput")
    t_bias4 = nc.dram_tensor("bias4", [128, 1], f32, kind="ExternalInput")
    t_bias0 = nc.dram_tensor("bias0", [128, 1], f32, kind="ExternalInput")
    t_expT128 = nc.dram_tensor("expT128", [128, 128], f32r, kind="ExternalInput")
    t_init128 = nc.dram_tensor("init128", [128, K * L], f32r, kind="ExternalInput")
    t_ident128 = nc.dram_tensor("ident128", [128, L], f32, kind="ExternalInput")
    t_b684 = nc.dram_tensor("b684", [128, BL], f32r, kind="ExternalInput")
    t_b468 = nc.dram_tensor("b468", [BL, 128], f32r, kind="ExternalInput")
    t_emout = nc.dram_tensor("emissions", [BL * S, L], f32, kind="ExternalOutput")
    t_racc = nc.dram_tensor("racc", [128, L], f32, kind="ExternalOutput")
    t_mass = nc.dram_tensor("mass", [BL, K], f32, kind="ExternalOutput")

    with tile.TileContext(nc) as tc:
        with (
            tc.tile_pool(name="consts", bufs=1) as consts,
            tc.tile_pool(name="big", bufs=1) as big,
            tc.tile_pool(name="work", bufs=2) as work,
            tc.tile_pool(name="emo", bufs=4) as emo,
            tc.tile_pool(name="ps", bufs=2, space=bass.MemorySpace.PSUM) as ps,
            tc.tile_pool(name="ps_s", bufs=2, space=bass.MemorySpace.PSUM) as ps_s,
        ):
            # ---- constants ----
            wt_sb = consts.tile([128, KH, L], f32r)
            nc.sync.dma_start(wt_sb, t_wt.rearrange("(k p) l -> p k l", p=128))
            bias4_sb = consts.tile([128, 1], f32)
            nc.sync.dma_start(bias4_sb, t_bias4[:, :])
            bias0_sb = consts.tile([128, 1], f32)
            nc.sync.dma_start(bias0_sb, t_bias0[:, :])
            expT_sb = consts.tile([128, 128], f32r)
            nc.sync.dma_start(expT_sb, t_expT128[:, :])
            ident_sb = consts.tile([128, L], f32)
            nc.sync.dma_start(ident_sb, t_ident128[:, :])
            b684_sb = consts.tile([128, BL], f32r)
            nc.sync.dma_start(b684_sb, t_b684[:, :])
            b468_sb = consts.tile([BL, 128], f32r)
            nc.sync.dma_start(b468_sb, t_b468[:, :])

            # bdall: 16 pre-zeroed blockdiag slots, filled by the normalize op
            bdall = big.tile([128, K, 128], f32)
            nc.gpsimd.memset(bdall, 0.0)

            # ---- hidden states, half-major: [128, KH, 2, BL, 256] ----
            hsT_sb = big.tile([128, KH, 2, BL, HS], f32r)
            for h in range(2):
                for k8 in range(KH):
                    nc.sync.dma_start(
                        hsT_sb[:, k8, h, :, :],
                        t_hsT[k8 * 128 : (k8 + 1) * 128, :]
                        .rearrange("p (b t) -> p b t", t=S)[:, :, h * HS : (h + 1) * HS],
                    )

            # ---- emissions: per (half, seq) psum [17, 256], bank-rotated ----
            w128 = big.tile([128, S], f32)
            em128 = big.tile([128, S], f32)
            for h in range(2):
                for b in range(BL):
                    pe_em = ps.tile([L, HS], f32, tag="em")
                    for k8 in range(KH):
                        nc.tensor.matmul(
                            pe_em,
                            wt_sb[:, k8, :],
                            hsT_sb[:, k8, h, b, :],
                            start=(k8 == 0),
                            stop=(k8 == KH - 1),
                        )
                    nc.scalar.activation(
                        w128[b * PB : b * PB + L, h * HS : (h + 1) * HS],
                        pe_em,
                        mybir.ActivationFunctionType.Exp,
                        bias=bias4_sb[b * PB : b * PB + L, :],
                    )
                    nc.vector.tensor_scalar_add(
                        em128[b * PB : b * PB + L, h * HS : (h + 1) * HS],
                        pe_em,
                        bias0_sb[b * PB : b * PB + L, :],
                    )

            # ---- emissions output: PE-transpose [17,128] -> [128,17] ----
            for b in range(BL if emout else 0):
                for q in range(S // 128):
                    ps_t = ps_s.tile([128, L], f32, tag="pa1")
                    nc.tensor.transpose(
                        ps_t,
                        em128[b * PB : b * PB + L, q * 128 : (q + 1) * 128],
                        ident_sb[b * PB : b * PB + L, :],
                        tile_position=(b * PB, 0),
                    )
                    ot = emo.tile([128, L], f32, tag="ot")
                    nc.vector.tensor_copy(ot, ps_t)
                    nc.sync.dma_start(
                        t_emout[b * S + q * 128 : b * S + (q + 1) * 128, :], ot
                    )

            # ---- Phase A: two independent half-scans (chunks 0-7 | 8-15) ----
            Rfins = []
            for h in range(2):
                Rx = big.tile([128, KHALF, L], f32r, tag=f"Ra{h}")
                Ry = big.tile([128, KHALF, L], f32r, tag=f"Rb{h}")
                nc.sync.dma_start(
                    Rx,
                    t_init128.rearrange("p (k x) -> p k x", x=L)[
                        :, h * KHALF : (h + 1) * KHALF, :
                    ],
                )
                R = [Rx, Ry]
                for c in range(C if scan else 0):
                    src, dst = R[c % 2], R[(c + 1) % 2]
                    # chunk kk (global k = h*KHALF+kk) applies w col k*C+c+1
                    last = h == 1 and c == C - 1
                    nk = KHALF - 1 if last else KHALF
                    pa = ps_s.tile([128, KHALF, L], f32, tag=f"pa{h}")
                    nc.tensor.matmul(pa[:, :, :], expT_sb[:, :], src[:, :, :])
                    base = w128[:, h * KHALF * C + c + 1 : h * KHALF * C + c + 2]
                    w_ap = bass.AP(
                        tensor=base.tensor,
                        offset=base.offset,
                        ap=[base.ap[0], [C, nk], [0, L]],
                    )
                    nc.vector.tensor_mul(dst[:, :nk, :], pa[:, :nk, :], w_ap)
                    if nk < KHALF:
                        nc.vector.tensor_copy(dst[:, nk:, :], src[:, nk:, :])
                Rfins.append(R[C % 2])

            # ---- mass + normalize into bdall ----
            s16 = work.tile([BL, K], f32, tag="s16")
            for h in range(2):
                pm = ps_s.tile([BL, KHALF, L], f32, tag=f"pa{h}")
                nc.tensor.matmul(pm, b684_sb, Rfins[h])
                nc.vector.reduce_sum(
                    s16[:, h * KHALF : (h + 1) * KHALF],
                    pm,
                    axis=mybir.AxisListType.X,
                    op=mybir.AluOpType.add,
                )
            nc.sync.dma_start(t_mass[:, :], s16)
            rec = work.tile([BL, K], f32, tag="rec")
            nc.vector.reciprocal(rec, s16)
            pr = ps_s.tile([128, K], f32, tag="pa0")
            nc.tensor.matmul(pr, b468_sb[:, :].bitcast(f32), rec[:, :])
            rec128 = work.tile([128, K], f32, tag="rec128")
            nc.vector.tensor_copy(rec128, pr)
            # write normalized chunk matrices straight into blockdiag slots
            for h in range(2):
                for b in range(BL):
                    rb = rec128[b * PB : b * PB + L, 0:1]
                    rec_ap = bass.AP(
                        tensor=rb.tensor,
                        offset=rb.offset + h * KHALF,
                        ap=[rb.ap[0], [1, KHALF], [0, L]],
                    )
                    nc.vector.tensor_mul(
                        bdall[b * PB : b * PB + L, h * KHALF : (h + 1) * KHALF,
                              b * PB : b * PB + L],
                        Rfins[h][b * PB : b * PB + L, :, :],
                        rec_ap,
                    )

            # ---- Phase B: Racc <- A_k @ Racc, k = K-1..0 ----
            r0 = work.tile([128, L], f32, tag="r0")
            r1 = work.tile([128, L], f32, tag="r1")
            nc.vector.tensor_copy(r0, ident_sb[:, :])
            rp = [r0, r1]
            for i, k in enumerate(range(K - 1, -1, -1) if phaseb else []):
                src, dst = rp[i % 2], rp[(i + 1) % 2]
                pb = ps_s.tile([128, L], f32, tag="pb")
                nc.tensor.matmul(pb, bdall[:, k, :], src[:, :])
                nc.vector.tensor_copy(dst, pb)
            nc.sync.dma_start(t_racc[:, :], rp[K % 2])

    nc.compile()
    return nc


def _host_constants(hs, W, b, start_transitions, transitions):
    """Per-core input maps (device consts + sharded hsT) and c1 offsets."""
    f32 = np.float32
    wt = np.ascontiguousarray(W.T).astype(f32)                      # [H, L]
    expT = np.exp(transitions.astype(np.float64)).astype(f32)
    expT128 = np.zeros((128, 128), f32)
    bias4 = np.zeros((128, 1), f32)
    bias0 = np.zeros((128, 1), f32)
    ident128 = np.zeros((128, L), f32)
    b684 = np.zeros((128, BL), f32)
    b468 = np.zeros((BL, 128), f32)
    for b_ in range(BL):
        r = b_ * PB
        expT128[r : r + L, r : r + L] = expT
        bias4[r : r + L, 0] = b - 4 * LN2
        bias0[r : r + L, 0] = b
        ident128[r : r + L, :] = np.eye(L, dtype=f32)
        b684[r : r + L, b_] = 1.0 / 16.0
        b468[b_, r : r + L] = 1.0
    bias4[:, 0] = np.where(bias4[:, 0] == 0.0, -4 * LN2, bias4[:, 0])

    em0 = (hs[:, 0, :].astype(f32) @ W.T.astype(f32) + b).astype(f32)   # [B, L]
    alpha1 = start_transitions[None, :].astype(f32) + em0
    c1 = alpha1.max(1)                                              # [B]
    v1 = np.exp((alpha1 - c1[:, None]).astype(f32)).astype(f32)

    in_maps = []
    for core in range(NCORES):
        sl = slice(core * BL, (core + 1) * BL)
        hsT = np.ascontiguousarray(
            hs[sl].reshape(BL * S, H).T
        ).astype(f32)                                               # [H, BL*S]
        init128 = np.zeros((128, K * L), f32)
        for b_ in range(BL):
            blk = init128[b_ * PB : b_ * PB + L]
            for k_ in range(1, K):
                blk[:, k_ * L : (k_ + 1) * L] = np.eye(L, dtype=f32)
            blk[:, 0:L] = np.diag(v1[core * BL + b_])
        in_maps.append(
            {
                "hsT": hsT,
                "wt": wt,
                "bias4": bias4,
                "bias0": bias0,
                "expT128": expT128,
                "init128": init128,
                "ident128": ident128,
                "b684": b684,
                "b468": b468,
            }
        )
    return in_maps, c1


def _run(inputs, trace=False):
    from concourse.bass_utils import run_bass_kernel_spmd

    hs = np.asarray(inputs["hidden_states"], np.float32)
    labels = np.asarray(inputs["labels"])
    W = np.asarray(inputs["W"], np.float32)
    b = np.asarray(inputs["b"], np.float32)
    st = np.asarray(inputs["start_transitions"], np.float32)
    en = np.asarray(inputs["end_transitions"], np.float32)
    T = np.asarray(inputs["transitions"], np.float32)

    if "nc" not in _cache:
        _cache["nc"] = _build_program()
    nc = _cache["nc"]

    in_maps, c1 = _host_constants(hs, W, b, st, T)
    res = run_bass_kernel_spmd(
        nc, in_maps, core_ids=list(range(NCORES)), trace=trace
    )

    # ---- host assembly (f64) ----
    em = np.concatenate(
        [r["emissions"].reshape(BL, S, L) for r in res.results], axis=0
    ).astype(np.float32)                                            # [B, S, L]
    racc_g = np.stack([r["racc"] for r in res.results], axis=0)     # [8, 128, L]
    racc = np.zeros((B, L, L), np.float64)
    for core in range(NCORES):
        for b_ in range(BL):
            racc[core * BL + b_] = racc_g[core, b_ * PB : b_ * PB + L, :]
    s16 = np.concatenate([r["mass"] for r in res.results], axis=0).astype(
        np.float64
    )                                                               # [B, K]

    vS = racc.sum(axis=1)                                           # [B, L]
    partition = (
        np.log((vS * np.exp(en.astype(np.float64))[None]).sum(1))
        + c1.astype(np.float64)
        + (S - 1) * 4 * LN2
        + np.log(s16).sum(1)
    )
    tags = labels.astype(np.int64)
    em64 = em.astype(np.float64)
    emit = np.take_along_axis(em64, tags[:, :, None], axis=2)[:, :, 0]
    trans_sc = T.astype(np.float64)[tags[:, :-1], tags[:, 1:]]
    numerator = (
        st.astype(np.float64)[tags[:, 0]]
        + emit[:, 0]
        + (trans_sc + emit[:, 1:]).sum(1)
        + en.astype(np.float64)[tags[:, -1]]
    )
    llh = numerator - partition
    loss = np.float32(-llh.mean())
    return (loss, em), res


def kernel(**inputs):
    (loss, em), _ = _run(inputs, trace=False)
    return loss, em
